# revision 1
# baseline (speedup 1.0000x reference)
"""Sliding-window causal self-attention (WINDOW=256) on 8 trn2 NeuronCores.

Sharding: 8 cores = 4 batch items x 2 sequence halves (1024 queries each).
Each core receives x pre-transposed with a 256-row key/value halo and
computes its output chunk fully independently; the host concatenates.

All-bf16 dataflow (fp8 was tried and rejected: ~2e-2 rel err).  The
schedule pipelines engines: attention for a head pair starts right
after its K/Q projection, with remaining projections interleaved
between score (S = K^T Q) and A*V matmul groups as PE filler; Exp and
Ln share one activation-table set (no table thrash); K/Q biases ride
the PSUM->SBUF copy as DVE tensor_scalar adds; A*V accumulates via
contiguous per-key-tile runs (12 matmuls/head instead of 28); input
DMAs stream on two HWDGE queues; output is bf16, widened on host.
"""

import numpy as np
import ml_dtypes

import concourse.bass as bass
import concourse.bacc as bacc
import concourse.bacc as bacc_mod
import concourse.mybir as mybir
from concourse.hw_specs import get_activation_tables
from concourse.tile import TileContext
from concourse.bass_utils import run_bass_kernel_spmd

F32 = mybir.dt.float32
BF16 = mybir.dt.bfloat16
FP8 = mybir.dt.float8e4
AF = mybir.ActivationFunctionType
OP = mybir.AluOpType
DR = mybir.MatmulPerfMode.DoubleRow

N_HEAD = 12
WINDOW = 256
B, T, C = 4, 2048, 768
HD = C // N_HEAD              # 64
TQ = 1024                     # queries per core
HALO = 256
ROWS = TQ + HALO              # 1280 rows of k/v per core
NCT = C // 128                # 6 contraction tiles
NKT = ROWS // 128             # 10 key tiles
WIN = 384                     # q-window width per key tile
SCALE = 1.0 / float(np.sqrt(HD))
SW = 64.0                     # fp8 weight pre-scale (host) -> rescale on copy
ISW = 1.0 / SW

# q-window start per key tile (compile-time, same on every core)
QS = [min(max(128 * (kt - 2), 0), TQ - WIN) for kt in range(NKT)]

_TABLES_PATCHED = False


def _patch_act_tables():
    """Make Exp and Ln resolve only to natural_log_exp_and_others so the
    table-load inserter picks one set for both (no per-head reloads)."""
    global _TABLES_PATCHED
    if _TABLES_PATCHED:
        return
    _TABLES_PATCHED = True

    def patched(arch):
        out = {}
        for name, fns in get_activation_tables(arch).items():
            fns = set(fns)
            if name != "natural_log_exp_and_others":
                fns.discard(AF.Exp)
                fns.discard(AF.Ln)
            out[name] = fns
        return out

    bacc_mod.get_activation_tables = patched


def _build_nc() -> bass.Bass:
    _patch_act_tables()
    nc = bacc.Bacc()

    wka_d = nc.dram_tensor("wka", [C, 384], BF16, kind="ExternalInput")
    wkb_d = nc.dram_tensor("wkb", [C, 384], BF16, kind="ExternalInput")
    wqa_d = nc.dram_tensor("wqa", [C, 384], BF16, kind="ExternalInput")
    wqb_d = nc.dram_tensor("wqb", [C, 384], BF16, kind="ExternalInput")
    wva_d = nc.dram_tensor("wva", [C, 384], BF16, kind="ExternalInput")
    wvb_d = nc.dram_tensor("wvb", [C, 384], BF16, kind="ExternalInput")
    wp_d = nc.dram_tensor("wp", [C, C], BF16, kind="ExternalInput")
    # bf16 x for the V projection (error there lands directly on the output)
    xb0_d = nc.dram_tensor("xb0", [C, 512], BF16, kind="ExternalInput")
    xb1_d = nc.dram_tensor("xb1", [C, 512], BF16, kind="ExternalInput")
    xb2_d = nc.dram_tensor("xb2", [C, 256], BF16, kind="ExternalInput")
    bkq_d = nc.dram_tensor("bkq", [128, 2 * NCT], F32, kind="ExternalInput")
    bv_d = nc.dram_tensor("bv64", [1, C], BF16, kind="ExternalInput")
    bp_d = nc.dram_tensor("bp64", [1, C], BF16, kind="ExternalInput")
    mask_d = nc.dram_tensor("mask", [128, NKT * WIN], BF16, kind="ExternalInput")
    out = nc.dram_tensor("out", [TQ, C], BF16, kind="ExternalOutput")

    with TileContext(nc) as tc:
        with (
            tc.tile_pool(name="persist", bufs=1) as pp,
            tc.tile_pool(name="work", bufs=3) as wkp,
            tc.tile_pool(name="et", bufs=4) as et_pool,
            tc.tile_pool(name="psA", bufs=2, space="PSUM") as psA,
            tc.tile_pool(name="psS", bufs=2, space="PSUM") as psS,
            tc.tile_pool(name="psY", bufs=2, space="PSUM") as psY,
        ):
            # ---- persistent SBUF tensors ----
            wka = pp.tile([128, NCT, 384], BF16)
            wkb = pp.tile([128, NCT, 384], BF16)
            wqa = pp.tile([128, NCT, 384], BF16)
            wqb = pp.tile([128, NCT, 384], BF16)
            wva = pp.tile([128, NCT, 384], BF16)
            wvb = pp.tile([128, NCT, 384], BF16)
            wp_sb = pp.tile([128, NCT, C], BF16)
            xb0 = pp.tile([128, NCT, 512], BF16)
            xb1 = pp.tile([128, NCT, 512], BF16)
            xb2 = pp.tile([128, NCT, 256], BF16)
            bkq_sb = pp.tile([128, 2 * NCT], F32)
            bv_sb = pp.tile([1, C], BF16)
            bp_sb = pp.tile([1, C], BF16)
            mask_sb = pp.tile([128, NKT, WIN], BF16)
            ones_sb = pp.tile([1, 512], BF16)

            qt_sb = pp.tile([128, NCT, TQ], BF16)     # Q_T: head dims on partitions
            kt_sb = pp.tile([128, NCT, ROWS], BF16)   # K_T
            v_sb = pp.tile([128, NKT, N_HEAD, 128], BF16)  # [V(64) | ones(64)]
            yn_sb = pp.tile([128, NCT, TQ], BF16)     # normalized Y_T

            # DMAs on two HWDGE queues: x on ACT, weights/biases/mask on SP
            nc.scalar.dma_start(xb0[:], xb0_d.rearrange("(t p) n -> p t n", p=128))
            nc.scalar.dma_start(xb1[:], xb1_d.rearrange("(t p) n -> p t n", p=128))
            nc.scalar.dma_start(xb2[:], xb2_d.rearrange("(t p) n -> p t n", p=128))
            nc.sync.dma_start(wka[:], wka_d.rearrange("(t p) n -> p t n", p=128))
            nc.sync.dma_start(bkq_sb[:], bkq_d[:])
            nc.sync.dma_start(wqa[:], wqa_d.rearrange("(t p) n -> p t n", p=128))
            nc.sync.dma_start(wva[:], wva_d.rearrange("(t p) n -> p t n", p=128))
            nc.sync.dma_start(bv_sb[:], bv_d[:])
            nc.sync.dma_start(mask_sb[:], mask_d.rearrange("p (k w) -> p k w", w=WIN))
            nc.sync.dma_start(wkb[:], wkb_d.rearrange("(t p) n -> p t n", p=128))
            nc.sync.dma_start(wqb[:], wqb_d.rearrange("(t p) n -> p t n", p=128))
            nc.sync.dma_start(wvb[:], wvb_d.rearrange("(t p) n -> p t n", p=128))
            nc.sync.dma_start(wp_sb[:], wp_d.rearrange("(t p) n -> p t n", p=128))
            nc.sync.dma_start(bp_sb[:], bp_d[:])
            nc.vector.memset(ones_sb[:], 1.0)
            # big constant fill on the otherwise-idle Pool engine; the V
            # copies overwrite the first 64 columns of each head block
            nc.gpsimd.memset(v_sb[:], 1.0)

            xchunks = ((xb0, 0, 512), (xb1, 512, 512), (xb2, 1024, 256))

            def w_at(wa, wb, ct):
                w, off = (wa, ct) if ct < 3 else (wb, ct - 3)
                return w, off * 128

            # K_T / Q_T projection for one output-channel tile ct: fp8
            # DoubleRow over k-tile pairs; 1/SW rescale + bias fused into
            # the PSUM->SBUF copy as a DVE tensor_scalar.
            def proj_kq(ct):
                for kind in (0, 1):  # 0 = K, 1 = Q
                    wa, wb = (wka, wkb) if kind == 0 else (wqa, wqb)
                    dst = kt_sb if kind == 0 else qt_sb
                    w, woff = w_at(wa, wb, ct)
                    bias = bkq_sb[:, kind * NCT + ct:kind * NCT + ct + 1]
                    for xt, x0, xw in xchunks:
                        if kind == 1:
                            lo = max(0, HALO - x0)
                            if lo >= xw:
                                continue
                            src0, ncols, d0 = lo, xw - lo, x0 + lo - HALO
                        else:
                            src0, ncols, d0 = 0, xw, x0
                        ps = psA.tile([128, 512], F32, tag="mm")
                        for c in range(NCT):
                            nc.tensor.matmul(
                                ps[:, :ncols],
                                w[:, c, woff:woff + 128],
                                xt[:, c, src0:src0 + ncols],
                                start=(c == 0),
                                stop=(c == NCT - 1),
                            )
                        nc.vector.tensor_scalar(
                            dst[:, ct, d0:d0 + ncols], ps[:, :ncols],
                            bias, None, OP.add,
                        )

            xbchunks = ((xb0, 0, 512), (xb1, 512, 512), (xb2, 1024, 256))

            # V projection for one 384-wide v-column half (6 heads), bf16
            def proj_v(half):
                wv = wva if half == 0 else wvb
                n0 = half * 384
                for r in range(NKT):
                    xt, x0, _ = xbchunks[min(r // 4, 2)]
                    src0 = r * 128 - x0
                    ps = psA.tile([128, 512], F32, tag="mm")
                    nc.tensor.matmul(
                        ps[:, :384], ones_sb[0:1, :128], bv_sb[0:1, n0:n0 + 384],
                        start=True, stop=False,
                    )
                    for c in range(NCT):
                        nc.tensor.matmul(
                            ps[:, :384],
                            xt[:, c, src0:src0 + 128],
                            wv[:, c, :384],
                            start=False,
                            stop=(c == NCT - 1),
                        )
                    h0 = half * 6
                    nc.vector.tensor_copy(
                        out=v_sb[:, r, h0:h0 + 6, 0:HD],
                        in_=ps[:, :384].rearrange("p (h d) -> p h d", d=HD),
                    )

            # attention scores + exp for one head -> et tile
            def attn_s(h):
                ct = h // 2
                p0 = (h % 2) * HD
                et = et_pool.tile([128, NKT, WIN], BF16, tag="et")
                for kt2 in range(0, NKT, 2):
                    ps_s = psS.tile([128, 2, 512], F32, tag="ss")
                    for j in range(2):
                        kt = kt2 + j
                        nc.tensor.matmul(
                            ps_s[:, j, :WIN],
                            kt_sb[p0:p0 + HD, ct, kt * 128:(kt + 1) * 128],
                            qt_sb[p0:p0 + HD, ct, QS[kt]:QS[kt] + WIN],
                            start=True, stop=True,
                        )
                    nc.scalar.activation(
                        et[:, kt2:kt2 + 2, :], ps_s[:, :, :WIN], AF.Exp,
                        scale=SCALE,
                    )
                nc.vector.tensor_tensor(et[:], et[:], mask_sb[:], OP.mult)
                return et

            # A*V + softmax denominators + normalize for one head
            def attn_av(h, et):
                ct = h // 2
                p0 = (h % 2) * HD
                for half in range(2):
                    ps_y = psY.tile([128, 512], F32, tag="y")
                    qb0 = half * 4
                    # per key tile, its 1-3 contributing query blocks in this
                    # half are contiguous -> one matmul run per kt (the 512-
                    # wide psY tile is exactly one PSUM bank, so any aligned
                    # sub-range write is legal)
                    runs = []
                    for kt in range(NKT):
                        qbs = [qb for qb in (kt - 2, kt - 1, kt)
                               if qb0 <= qb < qb0 + 4]
                        if qbs:
                            runs.append((kt, qbs[0], len(qbs)))
                    for i, (kt, qb, nrun) in enumerate(runs):
                        j0 = qb * 128 - QS[kt]
                        nc.tensor.matmul(
                            ps_y[:, (qb - qb0) * 128:(qb - qb0 + nrun) * 128],
                            v_sb[:, kt, h, :],
                            et[:, kt, j0:j0 + nrun * 128],
                            start=(i == 0),
                            stop=(i == len(runs) - 1),
                            skip_group_check=True,
                        )
                    # 1/D via exp(-ln(D)): Ln on rows 64-127, Exp(scale=-1)
                    # shift-copies down to rows 0-63.  Same ACT table set.
                    rln = wkp.tile([128, 512], F32, tag="rln")
                    rec = wkp.tile([HD, 512], F32, tag="rec")
                    nc.scalar.activation(rln[HD:128, :], ps_y[HD:128, :], AF.Ln)
                    nc.scalar.activation(rec[:, :], rln[HD:128, :], AF.Exp,
                                         scale=-1.0)
                    nc.vector.tensor_tensor(
                        yn_sb[p0:p0 + HD, ct, half * 512:(half + 1) * 512],
                        ps_y[0:HD, :],
                        rec[:, :],
                        OP.mult,
                    )

            def out_proj(qb):
                o_sb = wkp.tile([128, C], BF16, tag="osb")
                for n0 in (0, 384):
                    ps = psA.tile([128, 512], F32, tag="mm")
                    nc.tensor.matmul(
                        ps[:, :384], ones_sb[0:1, :128], bp_sb[0:1, n0:n0 + 384],
                        start=True, stop=False,
                    )
                    for c in range(NCT):
                        nc.tensor.matmul(
                            ps[:, :384],
                            yn_sb[:, c, qb * 128:(qb + 1) * 128],
                            wp_sb[:, c, n0:n0 + 384],
                            start=False,
                            stop=(c == NCT - 1),
                        )
                    nc.any.tensor_copy(out=o_sb[:, n0:n0 + 384], in_=ps[:, :384])
                nc.sync.dma_start(out[qb * 128:(qb + 1) * 128, :], o_sb[:])

            # ---- issue order: attention starts after K0/Q0; remaining
            # projections spread across head-pair blocks as PE filler ----
            proj_kq(0)

            ets = {}
            ets[0] = attn_s(0)
            ets[1] = attn_s(1)
            proj_v(0)
            attn_av(0, ets.pop(0))
            proj_kq(1)
            attn_av(1, ets.pop(1))

            ets[2] = attn_s(2)
            ets[3] = attn_s(3)
            proj_kq(2)
            attn_av(2, ets.pop(2))
            attn_av(3, ets.pop(3))

            ets[4] = attn_s(4)
            ets[5] = attn_s(5)
            proj_kq(3)
            attn_av(4, ets.pop(4))
            attn_av(5, ets.pop(5))

            ets[6] = attn_s(6)
            ets[7] = attn_s(7)
            proj_v(1)
            attn_av(6, ets.pop(6))
            proj_kq(4)
            attn_av(7, ets.pop(7))

            ets[8] = attn_s(8)
            ets[9] = attn_s(9)
            proj_kq(5)
            attn_av(8, ets.pop(8))
            ets[10] = attn_s(10)
            attn_av(9, ets.pop(9))
            ets[11] = attn_s(11)
            attn_av(10, ets.pop(10))
            attn_av(11, ets.pop(11))

            for qb in range(8):
                out_proj(qb)

    nc.compile()
    return nc


_NC_CACHE = []


def _get_nc() -> bass.Bass:
    if not _NC_CACHE:
        _NC_CACHE.append(_build_nc())
    return _NC_CACHE[0]


def _make_mask(half: int) -> np.ndarray:
    chunk_start = half * TQ
    p = np.arange(128)[:, None, None]
    kt = np.arange(NKT)[None, :, None]
    j = np.arange(WIN)[None, None, :]
    lk = 128 * kt + p
    qi = np.array(QS)[None, :, None] + j
    band = (qi >= lk - WINDOW) & (qi <= lk - 1)
    exists = (chunk_start - HALO + lk) >= 0
    m = (band & exists).astype(ml_dtypes.bfloat16)
    return m.reshape(128, NKT * WIN)


def build_in_maps(x, W_attn, b_attn, W_proj, b_proj):
    x = np.asarray(x, dtype=np.float32)
    W_attn = np.asarray(W_attn, dtype=np.float32)
    b_attn = np.asarray(b_attn, dtype=np.float32)
    W_proj = np.asarray(W_proj, dtype=np.float32)
    b_proj = np.asarray(b_proj, dtype=np.float32)

    bf = ml_dtypes.bfloat16
    f8 = ml_dtypes.float8_e4m3
    wq_h = W_attn[:, 0:C].astype(bf)
    wk_h = W_attn[:, C:2 * C].astype(bf)
    wv_h = W_attn[:, 2 * C:3 * C].astype(bf)
    wp_h = np.ascontiguousarray(W_proj).astype(bf)
    bq_h = b_attn[0:C].astype(np.float32)
    bk_h = b_attn[C:2 * C].astype(np.float32)
    # [128, 2*NCT]: [:, 0:6] = K bias per ct, [:, 6:12] = Q bias per ct
    bkq_h = np.concatenate(
        [bk_h.reshape(NCT, 128).T, bq_h.reshape(NCT, 128).T], axis=1
    ).astype(np.float32)
    bv_h = b_attn[2 * C:3 * C].reshape(1, C).astype(bf)
    bp_h = b_proj.reshape(1, C).astype(bf)
    masks = [_make_mask(0), _make_mask(1)]

    halves = {
        "wka": np.ascontiguousarray(wk_h[:, 0:384]),
        "wkb": np.ascontiguousarray(wk_h[:, 384:768]),
        "wqa": np.ascontiguousarray(wq_h[:, 0:384]),
        "wqb": np.ascontiguousarray(wq_h[:, 384:768]),
        "wva": np.ascontiguousarray(wv_h[:, 0:384]),
        "wvb": np.ascontiguousarray(wv_h[:, 384:768]),
    }

    in_maps = []
    for core in range(8):
        b, half = divmod(core, 2)
        start = half * TQ - HALO
        if start < 0:
            x_win = np.concatenate(
                [np.zeros((HALO, C), np.float32), x[b, 0:TQ]], axis=0)
        else:
            x_win = x[b, start:start + ROWS]
        x_tb = np.ascontiguousarray(x_win.T).astype(bf)
        in_maps.append({
            "xb0": np.ascontiguousarray(x_tb[:, 0:512]),
            "xb1": np.ascontiguousarray(x_tb[:, 512:1024]),
            "xb2": np.ascontiguousarray(x_tb[:, 1024:1280]),
            **halves,
            "wp": wp_h, "bkq": bkq_h, "bv64": bv_h, "bp64": bp_h,
            "mask": masks[half],
        })
    return in_maps


def kernel(x, W_attn, b_attn, W_proj, b_proj):
    in_maps = build_in_maps(x, W_attn, b_attn, W_proj, b_proj)
    nc = _get_nc()
    res = run_bass_kernel_spmd(nc, in_maps, list(range(8)))
    y = np.empty((B, T, C), dtype=np.float32)
    for core in range(8):
        b, half = divmod(core, 2)
        y[b, half * TQ:(half + 1) * TQ, :] = res.results[core]["out"].astype(
            np.float32)
    return y



# revision 20
# speedup vs baseline: 2.1619x; 2.1619x over previous
"""Sliding-window causal self-attention (WINDOW=256) on 8 trn2 NeuronCores.

Sharding: 8 cores = 4 batch items x 2 sequence halves (1024 queries each).
Each core receives x pre-transposed with a 256-row key/value halo and
computes its output chunk fully independently; the host concatenates.

v2 restructure vs the first working kernel:
- score windows trimmed to the true band extent per key tile
  (3072 instead of 3840 columns per head), with the per-head exp/mask
  buffer packed to 3072 columns;
- softmax denominators via DVE reciprocal_approx_fast (frees the
  Activation engine of the Ln/Exp round trip);
- no bias matmuls: V bias is folded into the projection bias on the
  host (softmax rows sum to 1), and the out projection is computed
  transposed (output channels on partitions) so its bias rides the
  PSUM->SBUF copy as a per-partition tensor_scalar add;
- V copies and out copies pinned to the idle Pool engine;
- score matmuls for the two heads of a pair interleaved kt-by-kt so
  their (64-contraction) matmuls land in different PE row groups and
  can overlap on hardware;
- x staged as one contiguous [128, 6, 1280] tile so K/Q projections
  stream arbitrary 512-wide windows (fewer, larger matmuls).
"""

import numpy as np
import ml_dtypes

import concourse.bass as bass
import concourse.bacc as bacc
import concourse.bacc as bacc_mod
import concourse.mybir as mybir
from concourse.hw_specs import get_activation_tables
from concourse.tile import TileContext
from concourse.bass_utils import run_bass_kernel_spmd

F32 = mybir.dt.float32
BF16 = mybir.dt.bfloat16
FP8 = mybir.dt.float8e4
AF = mybir.ActivationFunctionType
OP = mybir.AluOpType

N_HEAD = 12
WINDOW = 256
B, T, C = 4, 2048, 768
HD = C // N_HEAD              # 64
TQ = 1024                     # queries per core
HALO = 256
ROWS = TQ + HALO              # 1280 rows of k/v per core
NCT = C // 128                # 6 contraction tiles
NKT = ROWS // 128             # 10 key tiles
SW = 64.0                     # K/Q weight pre-scale (host, in wall)
SCALE = 1.0 / float(np.sqrt(HD))

# Per key tile kt: valid query window [QS2[kt], QS2[kt]+W[kt]) (local query
# coords).  Packed et layout pairs narrow edge tiles so exp can process two
# tiles of equal width in one instruction: order (0,9),(1,8),(2,3),...,(6,7).
QS2 = [0, 0, 0, 128, 256, 384, 512, 640, 768, 896]
WID = [128, 256, 384, 384, 384, 384, 384, 384, 256, 128]
_pack_order = [0, 9, 1, 8, 2, 3, 4, 5, 6, 7]
OFF = [0] * NKT
_acc = 0
for _kt in _pack_order:
    OFF[_kt] = _acc
    _acc += WID[_kt]
ET_W = _acc                   # 3072

_TABLES_PATCHED = False


def _patch_act_tables():
    """Make Exp resolve only to exp_and_others so the table-load inserter
    picks a single set (no per-head table reloads)."""
    global _TABLES_PATCHED
    if _TABLES_PATCHED:
        return
    _TABLES_PATCHED = True

    def patched(arch):
        out = {}
        for name, fns in get_activation_tables(arch).items():
            fns = set(fns)
            if name != "exp_and_others":
                fns.discard(AF.Exp)
            out[name] = fns
        return out

    bacc_mod.get_activation_tables = patched


def _build_nc() -> bass.Bass:
    _patch_act_tables()
    nc = bacc.Bacc()

    # Inputs consolidated into 3 tensors: per-execution dispatch overhead
    # through the PJRT proxy scales with argument count.
    # wall columns: wk[0:768] | wq[768:1536] | wv[1536:2304] | wp[2304:3072]
    #               | mask[3072:3584] (mask [128, 3072] stored as [768, 512])
    wall_d = nc.dram_tensor("wall", [C, 3584], BF16, kind="ExternalInput")
    xba_d = nc.dram_tensor("xba", [C, ROWS], BF16, kind="ExternalInput")
    # bias columns: K bias per ct [0:6] | Q bias per ct [6:12] | proj [12:18]
    bias_d = nc.dram_tensor("bias", [128, 3 * NCT], F32, kind="ExternalInput")
    # transposed output: [C, TQ]; host transposes back
    out = nc.dram_tensor("out", [C, TQ], BF16, kind="ExternalOutput")

    with TileContext(nc) as tc:
        with (
            tc.tile_pool(name="persist", bufs=1) as pp,
            tc.tile_pool(name="work", bufs=3) as wkp,
            tc.tile_pool(name="et", bufs=4) as et_pool,
            tc.tile_pool(name="psA", bufs=3, space="PSUM") as psA,
            tc.tile_pool(name="psS", bufs=3, space="PSUM") as psS,
            tc.tile_pool(name="psY", bufs=2, space="PSUM") as psY,
        ):
            # ---- persistent SBUF tensors ----
            wk8 = pp.tile([128, NCT, C], FP8)
            wqa = pp.tile([128, NCT, 384], BF16)
            wqb = pp.tile([128, NCT, 384], BF16)
            x8 = pp.tile([128, NCT, ROWS], FP8)
            wva = pp.tile([128, NCT, 384], BF16)
            wvb = pp.tile([128, NCT, 384], BF16)
            wp_sb = pp.tile([128, NCT, C], BF16)
            xall = pp.tile([128, NCT, ROWS], BF16)
            bias_sb = pp.tile([128, 3 * NCT], F32)
            mask_sb = pp.tile([128, ET_W], BF16)

            qt_sb = pp.tile([128, NCT, TQ], BF16)     # Q_T: head dims on partitions
            kt_sb = pp.tile([128, NCT, ROWS], BF16)   # K_T
            v_sb = pp.tile([128, NKT, N_HEAD, 128], BF16)  # [ones(64) | V(64)]
            yn_sb = pp.tile([128, NCT, TQ], BF16)     # normalized Y_T
            o_sb = pp.tile([128, NCT, TQ], BF16)      # out_T staging

            # DMAs on two HWDGE queues: x on ACT, weights/mask/bias on SP
            def wcol(c0, c1):
                return wall_d[:, c0:c1].rearrange("(t p) n -> p t n", p=128)

            nc.scalar.dma_start(xall[:, :, 0:512],
                                xba_d[:, 0:512].rearrange("(t p) n -> p t n",
                                                          p=128))
            nc.scalar.dma_start(xall[:, :, 512:1024],
                                xba_d[:, 512:1024].rearrange(
                                    "(t p) n -> p t n", p=128))
            nc.scalar.dma_start(xall[:, :, 1024:1280],
                                xba_d[:, 1024:1280].rearrange(
                                    "(t p) n -> p t n", p=128))
            nc.gpsimd.dma_start(wk8[:], wcol(0, 768))
            nc.gpsimd.dma_start(x8[:, :, 0:512],
                                xba_d[:, 0:512].rearrange("(t p) n -> p t n",
                                                          p=128))
            nc.gpsimd.dma_start(x8[:, :, 512:1024],
                                xba_d[:, 512:1024].rearrange(
                                    "(t p) n -> p t n", p=128))
            nc.gpsimd.dma_start(x8[:, :, 1024:1280],
                                xba_d[:, 1024:1280].rearrange(
                                    "(t p) n -> p t n", p=128))
            nc.sync.dma_start(bias_sb[:], bias_d[:])
            nc.sync.dma_start(wqa[:], wcol(768, 1152))
            nc.sync.dma_start(wqb[:], wcol(1152, 1536))
            nc.sync.dma_start(wva[:], wcol(1536, 1920))
            nc.sync.dma_start(
                mask_sb[:].rearrange("p (k c) -> p k c", c=512),
                wall_d[:, 3072:3584].rearrange("(p k) c -> p k c", p=128))
            nc.sync.dma_start(wvb[:], wcol(1920, 2304))
            nc.sync.dma_start(wp_sb[:], wcol(2304, 3072))
            # only the ones half of each head block; V copies fill the rest
            nc.gpsimd.memset(v_sb[:, :, :, 0:HD], 1.0)

            def w_at(wa, wb, ct):
                w, off = (wa, ct) if ct < 3 else (wb, ct - 3)
                return w, off * 128

            # (kind, x0, ncols): ordered so chunks needing late x arrive last
            KQ_CHUNKS = ((0, 0, 512), (0, 512, 512), (1, 256, 512),
                         (0, 1024, 256), (1, 768, 512))

            # K_T / Q_T projection for one output-channel tile ct; bias rides
            # the PSUM->SBUF copy as a DVE tensor_scalar add.
            def proj_kq(ct):
                for kind, x0, ncols in KQ_CHUNKS:
                    dst = kt_sb if kind == 0 else qt_sb
                    bias = bias_sb[:, kind * NCT + ct:kind * NCT + ct + 1]
                    d0 = x0 if kind == 0 else x0 - HALO
                    ps = psA.tile([128, 512], F32, tag="mm")
                    if kind == 0:
                        for cp in range(NCT // 2):
                            nc.tensor.matmul(
                                ps[:, :ncols],
                                wk8[:, 2 * cp:2 * cp + 2,
                                    ct * 128:(ct + 1) * 128],
                                x8[:, 2 * cp:2 * cp + 2, x0:x0 + ncols],
                                start=(cp == 0),
                                stop=(cp == NCT // 2 - 1),
                                perf_mode=mybir.MatmulPerfMode.DoubleRow,
                            )
                    else:
                        w, woff = w_at(wqa, wqb, ct)
                        for c in range(NCT):
                            nc.tensor.matmul(
                                ps[:, :ncols],
                                w[:, c, woff:woff + 128],
                                xall[:, c, x0:x0 + ncols],
                                start=(c == 0),
                                stop=(c == NCT - 1),
                            )
                    nc.vector.tensor_scalar(
                        dst[:, ct, d0:d0 + ncols], ps[:, :ncols],
                        bias, None, OP.add,
                    )

            # V projection for one 384-wide v-column half (6 heads), bf16;
            # copies land on the Pool engine.
            def proj_v(half):
                wv = wva if half == 0 else wvb
                h0 = half * 6
                for r in range(NKT):
                    ps = psA.tile([128, 512], F32, tag="mm")
                    for c in range(NCT):
                        nc.tensor.matmul(
                            ps[:, :384],
                            xall[:, c, r * 128:(r + 1) * 128],
                            wv[:, c, :384],
                            start=(c == 0),
                            stop=(c == NCT - 1),
                        )
                    nc.scalar.activation(
                        v_sb[:, r, h0:h0 + 6, HD:128],
                        ps[:, :384].rearrange("p (h d) -> p h d", d=HD),
                        AF.Copy,
                    )

            # scores + exp for the two heads of pair hp, interleaved kt-by-kt
            # so the K=64 matmuls land in PE row groups (0,0) and (64,0).
            def attn_s2(hp):
                ct = hp
                et0 = et_pool.tile([128, ET_W], BF16, tag="et")
                et1 = et_pool.tile([128, ET_W], BF16, tag="et")
                ets = (et0, et1)
                for kt in range(NKT):
                    w = WID[kt]
                    tiles = []
                    for j in (0, 1):
                        p0 = j * HD
                        ps_s = psS.tile([128, 512], F32, tag="ss")
                        nc.tensor.matmul(
                            ps_s[:, :w],
                            kt_sb[p0:p0 + HD, ct, kt * 128:(kt + 1) * 128],
                            qt_sb[p0:p0 + HD, ct, QS2[kt]:QS2[kt] + w],
                            start=True, stop=True,
                        )
                        tiles.append(ps_s)
                    for j in (0, 1):
                        nc.scalar.activation(
                            ets[j][:, OFF[kt]:OFF[kt] + w], tiles[j][:, :w],
                            AF.Exp, scale=SCALE / SW,
                        )
                for j in (0, 1):
                    nc.vector.tensor_tensor(ets[j][:], ets[j][:], mask_sb[:],
                                            OP.mult)
                return ets

            # A*V + denominators (ones columns) + normalize for one head
            def attn_av(h, et):
                ct = h // 2
                p0 = (h % 2) * HD
                for half in range(2):
                    ps_y = psY.tile([128, 512], F32, tag="y")
                    qb0 = half * 4
                    # runs split at the half-bank boundary (col 256) and
                    # ordered widest-first per side so each matmul's PSUM
                    # region is uniformly fresh or already-written (bank
                    # auto-zero is region-granular)
                    runs = []
                    for kt in range(NKT):
                        qbs = [qb for qb in (kt - 2, kt - 1, kt)
                               if qb0 <= qb < qb0 + 4 and 0 <= qb <= 7]
                        if not qbs:
                            continue
                        lo, n = qbs[0] - qb0, len(qbs)
                        for s0, s1 in ((max(lo, 0), min(lo + n, 2)),
                                       (max(lo, 2), min(lo + n, 4))):
                            if s1 > s0:
                                runs.append((kt, qb0 + s0, s1 - s0))
                    runs.sort(key=lambda r: ((r[1] - qb0) >= 2, -r[2]))
                    for i, (kt, qb, nrun) in enumerate(runs):
                        j0 = qb * 128 - QS2[kt] + OFF[kt]
                        nc.tensor.matmul(
                            ps_y[:, (qb - qb0) * 128:(qb - qb0 + nrun) * 128],
                            v_sb[:, kt, h, :],
                            et[:, j0:j0 + nrun * 128],
                            start=(i == 0),
                            stop=(i == len(runs) - 1),
                            skip_group_check=True,
                        )
                    # 1/D on DVE: the ones block puts D on rows 0-63
                    # (custom DVE ops require base partition 0); AV sits on
                    # rows 64-127 and the normalize multiply reads it with a
                    # base-shifted PSUM operand (HW-verified pattern).
                    rec = wkp.tile([128, 512], F32, tag="rec")
                    nc.vector.reciprocal_approx_fast(
                        out=rec[0:HD, :], in_=ps_y[0:HD, :])
                    nc.vector.tensor_tensor(
                        yn_sb[p0:p0 + HD, ct, half * 512:(half + 1) * 512],
                        ps_y[HD:128, :],
                        rec[0:HD, :],
                        OP.mult,
                    )

            # transposed out projection: stationary W_p tile, yn streams;
            # per-partition bias rides the copy; Pool does the copy.
            def out_proj(oc):
                ps_o0 = psA.tile([128, 512], F32, tag="mm")
                ps_o1 = psS.tile([128, 512], F32, tag="ss")
                pst = (ps_o0, ps_o1)
                for c in range(NCT):
                    for qh in (0, 1):
                        nc.tensor.matmul(
                            pst[qh][:, :],
                            wp_sb[:, c, oc * 128:(oc + 1) * 128],
                            yn_sb[:, c, qh * 512:(qh + 1) * 512],
                            start=(c == 0),
                            stop=(c == NCT - 1),
                        )
                for qh in (0, 1):
                    nc.vector.tensor_scalar(
                        o_sb[:, oc, qh * 512:(qh + 1) * 512], pst[qh][:, :],
                        bias_sb[:, 2 * NCT + oc:2 * NCT + oc + 1], None, OP.add,
                    )
                nc.sync.dma_start(out[oc * 128:(oc + 1) * 128, :],
                                  o_sb[:, oc, :])

            # ---- schedule ----
            proj_kq(0)
            proj_v(0)      # before scores: V copies drain while score MMs run
            for hp in range(6):
                e0, e1 = attn_s2(hp)
                if hp == 1:
                    proj_v(1)
                if hp < 5:
                    proj_kq(hp + 1)
                attn_av(2 * hp, e0)
                attn_av(2 * hp + 1, e1)
            for oc in range(NCT):
                out_proj(oc)

    nc.compile()
    return nc


_NC_CACHE = []


def _get_nc() -> bass.Bass:
    if not _NC_CACHE:
        _NC_CACHE.append(_build_nc())
    return _NC_CACHE[0]


def _make_mask(half: int) -> np.ndarray:
    """Packed band/existence mask matching the et layout."""
    chunk_start = half * TQ
    m = np.zeros((128, ET_W), dtype=np.float32)
    p = np.arange(128)[:, None]
    for kt in range(NKT):
        j = np.arange(WID[kt])[None, :]
        qi = QS2[kt] + j
        lk = 128 * kt + p
        band = (qi >= lk - WINDOW) & (qi <= lk - 1)
        exists = (chunk_start - HALO + lk) >= 0
        m[:, OFF[kt]:OFF[kt] + WID[kt]] = (band & exists).astype(np.float32)
    return m.astype(ml_dtypes.bfloat16)


def build_in_maps(x, W_attn, b_attn, W_proj, b_proj):
    x = np.asarray(x, dtype=np.float32)
    W_attn = np.asarray(W_attn, dtype=np.float32)
    b_attn = np.asarray(b_attn, dtype=np.float32)
    W_proj = np.asarray(W_proj, dtype=np.float32)
    b_proj = np.asarray(b_proj, dtype=np.float32)

    bf = ml_dtypes.bfloat16
    wq_h = W_attn[:, 0:C]
    wk_h = W_attn[:, C:2 * C]
    wv_h = W_attn[:, 2 * C:3 * C]
    bq_h = b_attn[0:C].astype(np.float32)
    bk_h = b_attn[C:2 * C].astype(np.float32)
    bv_h = b_attn[2 * C:3 * C].astype(np.float32)
    # bias: K per ct | Q per ct | proj per oc (V bias folded: softmax rows
    # sum to 1 so it passes through attention unchanged)
    bp_prime = b_proj + bv_h @ W_proj
    bias_h = np.concatenate(
        [64.0 * bk_h.reshape(NCT, 128).T, bq_h.reshape(NCT, 128).T,
         bp_prime.reshape(NCT, 128).T], axis=1).astype(np.float32)
    masks = [_make_mask(0), _make_mask(1)]

    walls = []
    for half in range(2):
        wall = np.concatenate(
            [64.0 * wk_h, wq_h, wv_h, W_proj,
             np.asarray(masks[half], dtype=np.float32).reshape(C, 512)],
            axis=1).astype(bf)
        walls.append(np.ascontiguousarray(wall))

    in_maps = []
    for core in range(8):
        b, half = divmod(core, 2)
        start = half * TQ - HALO
        if start < 0:
            x_win = np.concatenate(
                [np.zeros((HALO, C), np.float32), x[b, 0:TQ]], axis=0)
        else:
            x_win = x[b, start:start + ROWS]
        x_tb = np.ascontiguousarray(x_win.T.astype(bf))
        in_maps.append({
            "wall": walls[half], "xba": x_tb, "bias": bias_h,
        })
    return in_maps


def kernel(x, W_attn, b_attn, W_proj, b_proj):
    in_maps = build_in_maps(x, W_attn, b_attn, W_proj, b_proj)
    nc = _get_nc()
    res = run_bass_kernel_spmd(nc, in_maps, list(range(8)))
    y = np.empty((B, T, C), dtype=np.float32)
    for core in range(8):
        b, half = divmod(core, 2)
        y[b, half * TQ:(half + 1) * TQ, :] = res.results[core]["out"].T.astype(
            np.float32)
    return y
